# revision 23
# baseline (speedup 1.0000x reference)
"""CARAFE-Downsample Trainium2 kernel (nn_CARAFE_Downsample_85744727097492).

Full inputs -> full output. Internally shards across 8 NeuronCores:
core c handles batch b = c//2, output-row half h = c%2 (32 of 64 output rows).

v2 pipeline (DMA-bound; wire ~25us, PE ~16us hidden under it):
  1. compress: 1x1 conv C=256->64 as ONE fp8 DoubleRow matmul per 7-row
     block (both 128-channel k-tiles contract in one pass, 0.5 cyc/col).
  2. mask conv: 3x3 stride-2 conv 64->25 as 5 fp8 DoubleRow tap-PAIR
     matmuls (w2 scaled x256 into fp8 range, folded back via the exp
     activation scale; 9 taps paired as (0,1)(3,4)(6,7) dj-adjacent and
     (2,5)(8,zero) di-adjacent so every pair is a strided comp view;
     psum M padded 25->32 for the dual-fp8 ldweights ISA rule).
  3. softmax over the 25 taps: exp (ScalarE, x1/256 +b2), tap-sum via
     ones-matmul, reciprocal (VectorE), broadcast via ones-matmul, fused
     normalize -> mn bf16.
  4. reassembly (bf16, per wo-QUARTER q=(hw,nt2)): out accumulates
     xT_row(r)^T @ A_q where A_q [35, 5*512] holds normalized mask values
     banded at [u'=2wo'+j, i*512 + wo'*32 + ho].  A_q is scattered with one
     DMA through DRAM (flat addressing absorbs the diagonal; the 5*512
     i-block pitch merges the 25 mn partitions into a single AP dim), then
     gathered back (179KB/quarter - half the bytes of half-granular A).
     Quarter granularity pipelines gathers between partition-split xt
     chunk loads so reassembly starts as soon as its quarter is ready.

Mask channels are permuted (partition p <-> CARAFE tap (p%5, p//5), folded
into w2/b2 on the host) so each banded diagonal reads 5 contiguous partitions.

Output staged quarter-major [c, q*512 + ho*16 + wo]; host unshuffles.
"""
import os
import sys

sys.path.insert(0, "/opt/trn_rl_repo")

import numpy as np
import ml_dtypes

import concourse.bass as bass
import concourse.bacc as bacc
import concourse.tile as tile
from concourse import mybir
from concourse.bass_utils import run_bass_kernel_spmd
from concourse.tile_rust import add_dep_helper


def _dep(from_ins, to_ins, reason, sync=True):
    a = getattr(from_ins, "ins", from_ins)
    b = getattr(to_ins, "ins", to_ins)
    add_dep_helper(a, b, sync=sync, reason=reason)


BF16 = ml_dtypes.bfloat16
FP8 = ml_dtypes.float8_e4m3

# problem constants
B, C, H, W = 4, 256, 128, 128
COMP = 64
K, S = 5, 2
Ho, Wo = 64, 64
N_CORES = 8
W2_SCALE = 256.0

# per-core geometry
HR = 67            # x rows per core slice (padded grid)
WP = 132           # padded width
U = 67             # xt partitions per w-half
W_HALF = (67, 68)  # compress grid width per half
NP = (HR * W_HALF[0], HR * W_HALF[1])   # 4489, 4556
RBLK = [(t0, min(7, HR - t0)) for t0 in range(0, HR, 7)]
# mask-conv tap pairs: (tapA, tapB, axis) with axis 'v' = dj-adjacent
# (pair stride 1 col) or 'r' = di-adjacent (pair stride 1 row); tap 9 is
# the zero pad rider for tap 8.
MPAIRS = [(0, 1, "v"), (3, 4, "v"), (6, 7, "v"), (2, 5, "r"), (8, 9, "r")]
NQ = 4             # quarter q = hw*2 + nt2: wo in [16q,16q+16)
AQ_COLS = 5 * 512  # A-quarter free size; 5*512 pitch makes the scatter 1 DMA

_DT = mybir.dt


def _build_nc(debug=False):
    nc = bacc.Bacc("TRN2", target_bir_lowering=False, debug=False,
                   num_devices=N_CORES)
    dt = _DT
    # ---- DRAM I/O ----
    xn_d = [nc.dram_tensor(f"xn{hw}", [128, 2 * NP[hw]], dt.float8e4,
                           kind="ExternalInput") for hw in range(2)]
    xt_d = [nc.dram_tensor(f"xt{hw}", [U, HR * 256], dt.bfloat16,
                           kind="ExternalInput") for hw in range(2)]
    wpk_d = nc.dram_tensor("wpk", [128, 448], dt.float8e4,
                           kind="ExternalInput")
    bias_d = nc.dram_tensor("biasc", [64, 2], dt.float32, kind="ExternalInput")
    A_dram = [nc.dram_tensor(f"azer{q}", [35, AQ_COLS], dt.bfloat16,
                             kind="ExternalInput") for q in range(NQ)]
    out_d = nc.dram_tensor("out", [256, 2048], dt.bfloat16,
                           kind="ExternalOutput")
    if debug:
        comp_dbg = nc.dram_tensor("comp_dbg", [64, NP[0] + NP[1]], dt.float8e4,
                                  kind="ExternalOutput")
        mn_dbg = nc.dram_tensor("mn_dbg", [25, 2048], dt.bfloat16,
                                kind="ExternalOutput")
        A_dbg = nc.dram_tensor("A_dbg", [35, NQ * AQ_COLS], dt.bfloat16,
                               kind="ExternalOutput")

    from contextlib import ExitStack
    with tile.TileContext(nc) as tc, ExitStack() as es:
        cpool = es.enter_context(tc.tile_pool(name="consts", bufs=1))
        bigp = es.enter_context(tc.tile_pool(name="big", bufs=1))
        spool = es.enter_context(tc.tile_pool(name="small", bufs=1))
        ps_c = es.enter_context(tc.tile_pool(name="ps_c", bufs=2, space="PSUM"))
        ps_x = es.enter_context(tc.tile_pool(name="ps_x", bufs=1, space="PSUM"))
        ps_r = es.enter_context(tc.tile_pool(name="ps_r", bufs=1, space="PSUM"))
        ps_o = es.enter_context(tc.tile_pool(name="ps_o", bufs=3, space="PSUM"))

        # ---- const loads (2 DMAs) + on-chip ones/zeros ----
        wpk = cpool.tile([128, 448], dt.float8e4, tag="wpk")
        nc.sync.dma_start(wpk[:], wpk_d.ap())
        bias = cpool.tile([64, 2], dt.float32, tag="bias")
        nc.sync.dma_start(bias[:], bias_d.ap())
        o25c = cpool.tile([25, 1], dt.bfloat16, tag="o25c")
        nc.vector.memset(o25c[:], 1.0)
        o128r = cpool.tile([1, 128], dt.bfloat16, tag="o128r")
        nc.vector.memset(o128r[:], 1.0)
        zer = cpool.tile([1, 512], dt.bfloat16, tag="zer")
        nc.vector.memset(zer[:], 0.0)
        b1s = bias[0:64, 0:1]
        b2s = bias[0:25, 1:2]

        # ---- big input loads ----
        xn = []
        for hw in range(2):
            t = bigp.tile([128, 2 * NP[hw]], dt.float8e4, tag=f"xn{hw}",
                          name=f"xn{hw}")
            mid = 2 * 35 * W_HALF[hw]
            nc.sync.dma_start(t[:, :mid], xn_d[hw].ap()[:, :mid])
            nc.sync.dma_start(t[:, mid:], xn_d[hw].ap()[:, mid:])
            xn.append(t)
        # per-quarter xt slabs (base partition 0 for the PE tile alignment)
        xtq = [bigp.tile([35, HR * 256], dt.bfloat16, tag=f"xtq{q}",
                         name=f"xtq{q}") for q in range(NQ)]

        XTMID = 34 * 256   # r-chunk split: matmuls r<34 only need chunk a

        def load_xt(q, part):
            hw, nt2 = q // 2, q % 2
            sl = slice(0, XTMID) if part == 0 else slice(XTMID, HR * 256)
            nc.sync.dma_start(xtq[q][:, sl],
                              xt_d[hw].ap()[32 * nt2: 32 * nt2 + 35, sl])

        comp = [bigp.tile([64, NP[hw]], dt.float8e4, tag=f"comp{hw}",
                          name=f"comp{hw}") for hw in range(2)]
        e_sb = [spool.tile([25, 1024], dt.bfloat16, tag=f"e{hw}", name=f"e{hw}")
                for hw in range(2)]
        r_sb = [spool.tile([1, 1024], dt.bfloat16, tag=f"r{hw}", name=f"r{hw}")
                for hw in range(2)]
        A_sb = [spool.tile([35, AQ_COLS], dt.bfloat16, tag=f"A{q}",
                           name=f"Asb{q}") for q in range(NQ)]
        osb = [spool.tile([128, 2048], dt.bfloat16, tag=f"osb{cc}",
                          name=f"osb{cc}") for cc in range(2)]

        anchor = [None] * NQ
        scat = [None] * NQ
        w1v = wpk[:, 0:128].rearrange("k (t m) -> k t m", t=2)

        def do_compress(hw):
            wh = W_HALF[hw]
            for bi, (t0, nr) in enumerate(RBLK):
                n = nr * wh
                ps = ps_c.tile([64, 512], dt.float32, tag="cps", name="cps")
                rhs = xn[hw][:, 2 * t0 * wh: 2 * (t0 + nr) * wh].rearrange(
                    "k (t n) -> k t n", t=2)
                nc.tensor.matmul(ps[:, :n], w1v, rhs,
                                 perf_mode=mybir.MatmulPerfMode.DoubleRow,
                                 start=True, stop=True)
                dsl = comp[hw][:, t0 * wh: t0 * wh + n]
                if bi % 2 == 0:
                    nc.scalar.activation(dsl, ps[:, :n],
                                         mybir.ActivationFunctionType.Identity,
                                         bias=b1s, scale=1.0)
                else:
                    nc.vector.tensor_scalar_add(dsl, ps[:, :n], b1s)

        def do_mask(hw, nt2):
            wh = W_HALF[hw]
            c0 = 32 * nt2 + 1
            comp_v = comp[hw][:].rearrange("k (r v) -> k r v", v=wh)
            lg = ps_x.tile([32, 512], dt.float32, tag="lg", name="lg")
            for p, (ta, tb, axis) in enumerate(MPAIRS):
                di, dj = ta // 3, ta % 3
                if axis == "v":
                    # taps (di,dj),(di,dj+1): [k, 2, 16v, 32r]
                    sl = comp_v[:, di + 1: di + 65: 2, c0 + dj: c0 + dj + 32]
                    rhs = sl.rearrange("k r (v two) -> k two v r", two=2)
                else:
                    # taps (di,dj),(di+1,dj): [k, 2, 16v, 32r]
                    sl = comp_v[:, di + 1: di + 65,
                                c0 + dj: c0 + dj + 32: 2]
                    rhs = sl.rearrange("k (r two) v -> k two v r", two=2)
                lhsT = wpk[0:64, 128 + 64 * p: 128 + 64 * (p + 1)].rearrange(
                    "k (t m) -> k t m", t=2)
                nc.tensor.matmul(lg[:, :], lhsT, rhs,
                                 perf_mode=mybir.MatmulPerfMode.DoubleRow,
                                 start=(p == 0), stop=(p == 4))
            return lg

        def do_exp(hw, nt2, lg):
            sl = slice(nt2 * 512, (nt2 + 1) * 512)
            nc.scalar.activation(e_sb[hw][:, sl], lg[0:25, :],
                                 mybir.ActivationFunctionType.Exp,
                                 bias=b2s, scale=float(1.0 / W2_SCALE))

        def do_sum(hw, nt2):
            # tap-sum + reciprocal; only consumed by the evac-time rb128
            # broadcast, so this hangs OFF the scatter critical path
            sl = slice(nt2 * 512, (nt2 + 1) * 512)
            sps = ps_x.tile([1, 512], dt.float32, tag="sps", name="sps")
            nc.tensor.matmul(sps[:, :], o25c[:], e_sb[hw][:, sl])
            with nc.allow_low_precision("softmax denom 1/s in bf16"):
                nc.vector.reciprocal(r_sb[hw][:, sl], sps[:, :])

        rb = [None] * NQ

        def do_rb(q):
            # anchor ties the PE queue to the A gather (bands use raw APs);
            # it scribbles into the rb buffer, then the rb broadcast
            # (start=True) overwrites the whole bank
            hw, nt2 = q // 2, q % 2
            sl = slice(nt2 * 512, (nt2 + 1) * 512)
            rb[q] = ps_r.tile([128, 512], dt.float32, tag="rb", name="rb")
            anchor[q] = nc.tensor.matmul(rb[q][0:1, 0:32], A_sb[q][0:1, 0:1],
                                         A_sb[q][0:1, 0:32])
            nc.tensor.matmul(rb[q][:, :], o128r[:], r_sb[hw][:, sl])

        def do_scatter(q):
            hw, nt2 = q // 2, q % 2
            src = e_sb[hw][:, nt2 * 512:(nt2 + 1) * 512].rearrange(
                "t (w h) -> t w h", h=32)
            dst = bass.AP(A_dram[q], 0,
                          [[512, 25], [2 * AQ_COLS + 32, 16], [1, 32]])
            scat[q] = nc.sync.dma_start(dst, src)

        def do_gather(q):
            ld = nc.sync.dma_start(A_sb[q][:], A_dram[q].ap())
            _dep(ld, scat[q], "A scatter before load")

        def do_reassembly(q):
            for cc in range(2):
                ops = ps_o.tile([128, 512], dt.float32, tag="ops", name="ops")
                # claim + zero the bank so banded matmuls accumulate in any
                # order (per-element first-touch semantics)
                nc.tensor.matmul(ops[:, :], zer[0:1, 0:128],
                                 zer[0:1, 0:512], start=True, stop=False)
                work = []
                for r in range(HR):
                    ho_lo = max(0, (r - 1) // 2)
                    ho_hi = min(31, (r + 2) // 2)
                    if ho_lo <= ho_hi:
                        work.append((r, ho_lo, ho_hi - ho_lo + 1))
                n_mm = len(work)
                for mm, (r, ho_lo, npair) in enumerate(work):
                    lhsT = xtq[q][0:35,
                                  r * 256 + cc * 128: r * 256 + cc * 128 + 128]
                    i_hi = r - 2 * ho_lo + 2
                    a_ap = A_sb[q][:]
                    # A flat: u'*2560 + i*512 + wo*32 + ho; consecutive
                    # (ho+1, i-2) pairs step by -1023
                    rhs = bass.AP(
                        a_ap.tensor, a_ap.offset + i_hi * 512 + ho_lo,
                        [[AQ_COLS, 35], [-1023, npair], [32, 16]],
                    )
                    mi = nc.tensor.matmul(
                        ops[:, ho_lo * 16: (ho_lo + npair) * 16],
                        lhsT, rhs, start=False, stop=(mm == n_mm - 1),
                    )
                    _dep(mi, anchor[q], "A load before reassembly mm",
                         sync=False)
                # evac with the deferred softmax normalization: out = ops * r
                # (rb broadcasts 1/sum to 128 partitions), quarter-major slot
                dsl = osb[cc][:, q * 512:(q + 1) * 512]
                eng = nc.vector if (q + cc) % 2 == 0 else nc.gpsimd
                eng.tensor_tensor(dsl, ops[:], rb[q][:, :],
                                  op=mybir.AluOpType.mult)

        def do_out(q):
            for cc in range(2):
                nc.sync.dma_start(
                    out_d.ap()[cc * 128:(cc + 1) * 128,
                               q * 512:(q + 1) * 512],
                    osb[cc][:, q * 512:(q + 1) * 512])

        # ---- issue order: per-engine queue order == program order ----
        # The scatters gate only on their exp (ACT) now; sums/recips/rb
        # broadcasts run later, off the wire-critical path.
        load_xt(0, 0)
        load_xt(0, 1)

        do_compress(0)
        lg00 = do_mask(0, 0)
        do_exp(0, 0, lg00)
        lg01 = do_mask(0, 1)
        do_exp(0, 1, lg01)

        do_scatter(0)
        do_scatter(1)
        do_gather(0)
        do_gather(1)

        do_sum(0, 0)
        do_sum(0, 1)

        do_compress(1)
        lg10 = do_mask(1, 0)
        do_exp(1, 0, lg10)
        lg11 = do_mask(1, 1)
        do_exp(1, 1, lg11)

        load_xt(1, 0)
        load_xt(1, 1)
        load_xt(2, 0)
        load_xt(2, 1)
        do_scatter(2)
        do_scatter(3)
        do_gather(2)
        do_gather(3)
        load_xt(3, 0)
        load_xt(3, 1)

        do_sum(1, 0)
        do_sum(1, 1)

        for q in range(NQ):
            do_rb(q)
            do_reassembly(q)
        for q in range(NQ):
            do_out(q)

        if debug:
            nc.sync.dma_start(comp_dbg.ap()[:, :NP[0]], comp[0][:])
            nc.sync.dma_start(comp_dbg.ap()[:, NP[0]:], comp[1][:])
            for hw in range(2):
                nc.sync.dma_start(mn_dbg.ap()[:, hw * 1024:(hw + 1) * 1024],
                                  mn_sb[hw][:])
            for q in range(NQ):
                nc.sync.dma_start(
                    A_dbg.ap()[:, q * AQ_COLS:(q + 1) * AQ_COLS], A_sb[q][:])

    nc.compile()
    return nc


_NC_CACHE = {}


def _get_nc(debug=False):
    key = bool(debug)
    if key not in _NC_CACHE:
        _NC_CACHE[key] = _build_nc(debug=key)
    return _NC_CACHE[key]


# host-side tap order matching MPAIRS: w2t block index -> conv tap
W2T_TAPS = []
for ta, tb, _ax in MPAIRS:
    W2T_TAPS.append(ta)
    W2T_TAPS.append(tb)   # tap 9 == zero block


def _host_prep(x, w1, b1, w2, b2):
    """Build the 8 per-core input maps."""
    xp = np.pad(x, ((0, 0), (0, 0), (2, 2), (2, 2)))
    # wpk: [128, 448] fp8 = w1t [128, 2x64] | w2t [64, 10x32] (rows 0:64)
    wpk = np.zeros((128, 448), dtype=np.float32)
    w1f = w1[:, :, 0, 0]                      # [64, 256]
    for t in range(2):
        wpk[:, t * 64:(t + 1) * 64] = w1f[:, t * 128:(t + 1) * 128].T
    # permute mask channels: device partition p holds CARAFE tap
    # (i, j) = (p % 5, p // 5), i.e. channel (p%5)*5 + p//5
    perm = np.array([(p % 5) * 5 + p // 5 for p in range(25)])
    w2p = w2[perm] * W2_SCALE                 # [25, 64, 3, 3]
    for blk, tap in enumerate(W2T_TAPS):
        if tap > 8:
            continue
        di, dj = tap // 3, tap % 3
        wpk[0:64, 128 + blk * 32:128 + blk * 32 + 25] = w2p[:, :, di, dj].T
    wpk8 = wpk.astype(FP8)
    biasc = np.zeros((64, 2), dtype=np.float32)
    biasc[0:64, 0] = b1
    biasc[0:25, 1] = b2[perm]
    azer = np.zeros((35, AQ_COLS), dtype=BF16)
    in_maps = []
    for core in range(N_CORES):
        b, h = core // 2, core % 2
        xs = xp[b, :, 64 * h:64 * h + HR, :]            # (256, 67, 132)
        xs8 = xs.astype(FP8)
        xnl = []
        for hw in range(2):
            v0 = 0 if hw == 0 else 64
            wh = W_HALF[hw]
            sl = xs8[:, :, v0:v0 + wh]                  # (256, 67, wh)
            parts = []
            for (t0, nr) in RBLK:
                blk = sl[:, t0:t0 + nr, :].reshape(2, 128, nr * wh)
                parts.append(np.concatenate([blk[0], blk[1]], axis=1))
            xnl.append(np.concatenate(parts, axis=1))   # (128, 2*NP)
        xtf = np.ascontiguousarray(xs.transpose(2, 1, 0))  # (132, 67, 256)
        xt0 = xtf[0:U].reshape(U, HR * 256)
        xt1 = xtf[64:64 + U].reshape(U, HR * 256)
        m = {
            "xn0": np.ascontiguousarray(xnl[0]),
            "xn1": np.ascontiguousarray(xnl[1]),
            "xt0": np.ascontiguousarray(xt0).astype(BF16),
            "xt1": np.ascontiguousarray(xt1).astype(BF16),
            "wpk": wpk8, "biasc": biasc,
        }
        for q in range(NQ):
            m[f"azer{q}"] = azer
        in_maps.append(m)
    return in_maps


def kernel(x, w1, b1, w2, b2):
    x = np.asarray(x, dtype=np.float32)
    w1 = np.asarray(w1, dtype=np.float32)
    b1 = np.asarray(b1, dtype=np.float32)
    w2 = np.asarray(w2, dtype=np.float32)
    b2 = np.asarray(b2, dtype=np.float32)
    debug = bool(int(os.environ.get("KDBG", "0")))
    nc = _get_nc(debug=debug)
    in_maps = _host_prep(x, w1, b1, w2, b2)
    res = run_bass_kernel_spmd(nc, in_maps, core_ids=list(range(N_CORES)))
    out = np.empty((B, C, Ho, Wo), dtype=np.float32)
    for core in range(N_CORES):
        b, h = core // 2, core % 2
        o = res.results[core]["out"].astype(np.float32)
        # staged [256, q*512 + ho*16 + wo] -> [256, 32, 64]
        o = o.reshape(256, 4, 32, 16).transpose(0, 2, 1, 3).reshape(256, 32, 64)
        out[b, :, 32 * h:32 * h + 32, :] = o
    if debug:
        kernel._dbg = res.results
    return out


if __name__ == "__main__":
    rng = np.random.default_rng(0)
    x = rng.standard_normal((B, C, H, W), dtype=np.float32)
    w1 = (rng.standard_normal((COMP, C, 1, 1), dtype=np.float32) / np.sqrt(C))
    b1 = np.zeros(COMP, np.float32)
    w2 = rng.standard_normal((25, COMP, 3, 3), dtype=np.float32) * 0.001
    b2 = np.zeros(25, np.float32)
    out = kernel(x, w1, b1, w2, b2)
    print("out", out.shape, out.dtype, float(np.abs(out).mean()))
